# revision 22
# baseline (speedup 1.0000x reference)
"""GAT (2-layer, PyG-style GATConv) on 8 Trainium2 NeuronCores. v2

Strategy (dst-sharded, quad-packed stream gather):
- Nodes sharded by dst across 8 cores (12500 each); edges partitioned by dst
  core; segment-softmax + weighted aggregation local per dst shard.
- Node table packs FOUR nodes per 256B row ([4 x 16 bf16 h | 4 x fp32 a_src |
  pad]); a per-slot additive mask (0 / ln(mult) / -1e30) selects the sub-row
  inside the segment softmax (also handling duplicate edges exactly); exp()
  runs without max-subtraction (|e| <~ 30, safe in fp32).
- Dst tiles are degree-sorted then FFD bin-packed into GROUPS of <= 64
  columns (8192 idxs) and relabeled in group order. Each group is ONE
  fixed-size SWDGE gather instruction (constant idx-count register, -1 tail
  idxs skipped by ucode) -> the Pool engine runs a clean stream of maximal
  gathers with zero per-tile register traffic (measured ~2.1 ns/idx vs 4.56
  in the per-tile variant).
- Per-tile epilogue: Prelu(a_s + a_d) + mask -> exp (ACT, accum gives the
  denominator); weighted h sum via a halving fold tree (contiguous DVE adds
  only; the old strided tensor_reduce cost ~10x per element).
- 3 SPMD launches: transform (x@W1 + scores, host pre-tiled so DMA lines are
  1KB contiguous) / layer-1 aggregation / layer-2 aggregation + classifier +
  log_softmax. The gather schedule is identical for both layers.
"""

import numpy as np

import concourse.ap_utils as ap_utils
import concourse.bacc as bacc
import concourse.bass as bass
import concourse.mybir as mybir
from concourse.bass import round_up_to_multiple
from concourse.bass_utils import run_bass_kernel_spmd
from concourse.masks import make_identity
from concourse.tile import TileContext

P = 128
NCORES = 8
N = 100000
F_IN = 512
HID = 16
C_OUT = 32
NEG_SLOPE = 0.2
NROW = N // 4          # 25000 quad rows
ROWW = 128             # bf16 elems per table row (256B)
SH = N // NCORES       # nodes per core
T_TILES = (SH + P - 1) // P
SHP = T_TILES * P      # padded shard size (12544)
KC = F_IN // P
NEG_BIG = -1.0e30
GCOLS = 64             # columns per gather group (8192 idxs)
GIDX = GCOLS * P
GS = 7                 # tiles per output batch

FP = mybir.dt.float32
BF = mybir.dt.bfloat16
I16 = mybir.dt.int16
I32 = mybir.dt.int32


def _my_dma_gather(gp, out_ap, in_ap, idxs_ap, num_idxs, num_idxs_reg,
                   elem_size, elem_step, queue_num):
    """BassGpSimd.dma_gather (non-transpose, DRAM source) without the
    256B-elem_size restriction and with a runtime num_idxs register; the row
    stride (elem_step) must still be a multiple of 256B."""
    assert idxs_ap.dtype == I16
    assert in_ap.dtype == out_ap.dtype
    assert in_ap.space == bass.MemorySpace.DRAM
    assert idxs_ap.space == bass.MemorySpace.SBUF
    assert out_ap.space == bass.MemorySpace.SBUF
    assert ap_utils.ap_is_contiguous(out_ap.ap[1:])
    assert ap_utils.ap_is_contiguous(idxs_ap.ap[1:])
    assert in_ap.ap[-1][1] == out_ap.ap[-1][1] == elem_size
    assert out_ap.ap[0][1] * out_ap.ap[1][1] == round_up_to_multiple(num_idxs, 128)
    assert in_ap.ap[0][0] == elem_step
    stride_bytes = elem_step * mybir.dt.size(in_ap.dtype)
    assert stride_bytes % 256 == 0 and stride_bytes // 256 < 256
    _in_ap = gp.lower_ap_dma(in_ap, for_custom_bir_dma=True)
    _idxs_ap = gp.lower_ap(idxs_ap)
    _out_ap = gp.lower_ap(out_ap)
    return gp.add_instruction(
        mybir.InstDMAGatherAnt(
            name=gp.bass.get_next_instruction_name(),
            ins=[*_in_ap, _idxs_ap, gp.lower_val_access(gp.to_reg(num_idxs_reg))],
            outs=[_out_ap],
            transpose=False,
            num_idxs=num_idxs,
            elem_size=elem_size,
            stride_bytes_256=stride_bytes // 256,
            gen_mode=0,
            single_packet=False,
            queue_num=queue_num,
        )
    )


# ---------------------------------------------------------------------------
# Host-side preprocessing
# ---------------------------------------------------------------------------

def _wrap_idx(local_idx):
    """Wrap an int16 index list [M] (M % 128 == 0) into the SWDGE layout
    [128, M//16]: idx i at partition i%16, col i//16, replicated x8."""
    M = local_idx.shape[0]
    w = local_idx.reshape(M // 16, 16).T.astype(np.int16)  # [16, M//16]
    return np.tile(w, (8, 1))


class _Sched:
    pass


def _build_schedule(edge_index):
    """Shared (both layers) gather schedule.

    Tiles are degree-sorted, FFD bin-packed into groups of <= GCOLS columns,
    then relabeled so group members are consecutive. Per core, one combined
    int16 tensor [128, CW] holds per group: [idx GCOLS*8 | mask GCOLS*8 |
    a_dst 2*T_g] (a_dst filled per layer later).
    """
    src = np.asarray(edge_index[0], dtype=np.int64)
    dst = np.asarray(edge_index[1], dtype=np.int64)
    loops = np.arange(N, dtype=np.int64)
    src = np.concatenate([src, loops])
    dst = np.concatenate([dst, loops])
    core = dst // SH

    per_core = []
    for c in range(NCORES):
        m = core == c
        s_c = src[m]
        d_loc = dst[m] - c * SH
        quad = s_c >> 2
        sub = (s_c & 3).astype(np.int64)
        # slots: unique (d_loc, quad); per-(slot, sub) multiplicity
        ekey = (d_loc * NROW + quad) * 4 + sub
        uk, ucnt = np.unique(ekey, return_counts=True)
        skey = uk >> 2
        usub = (uk & 3).astype(np.int64)
        slot_ids, slot_inv = np.unique(skey, return_inverse=True)
        nslots = len(slot_ids)
        slot_d = slot_ids // NROW
        slot_q = (slot_ids % NROW).astype(np.int64)
        mask = np.full((nslots, 4), NEG_BIG, np.float32)
        mask[slot_inv, usub] = np.log(ucnt).astype(np.float32)
        deg = np.bincount(slot_d, minlength=SH)
        per_core.append((slot_d, slot_q, mask, deg))

    # shared degree-sorted tiling; K[t] = max over cores
    orders = [np.argsort(-pc[3], kind="stable").astype(np.int64)
              for pc in per_core]
    Kc = np.zeros((NCORES, T_TILES), np.int64)
    for c in range(NCORES):
        deg = per_core[c][3]
        ds = deg[orders[c]]
        grid = np.concatenate([ds, np.zeros(SHP - SH, np.int64)]).reshape(T_TILES, P)
        Kc[c] = grid.max(axis=1)
    K = np.maximum(Kc.max(axis=0), 1)
    assert int(K.max()) <= GCOLS

    # FFD bin packing of tiles into groups of <= GCOLS columns
    order_t = np.argsort(-K, kind="stable")
    bins = []  # [used_cols, [old_tile,...]]
    for t in order_t:
        kt = int(K[t])
        for b in bins:
            if b[0] + kt <= GCOLS:
                b[0] += kt
                b[1].append(int(t))
                break
        else:
            bins.append([kt, [int(t)]])
    perm = np.array([t for b in bins for t in b[1]], np.int64)  # new -> old
    K = K[perm]
    gsizes = [len(b[1]) for b in bins]
    NG = len(bins)

    s = _Sched()
    s.K = K
    s.NG = NG
    s.gsizes = np.array(gsizes, np.int64)
    s.gid = np.repeat(np.arange(NG), s.gsizes)          # tile -> group
    s.gstart = np.concatenate([[0], np.cumsum(s.gsizes)])  # first tile of grp
    # column offset of each tile inside its group
    coff_t = np.zeros(T_TILES, np.int64)
    for g in range(NG):
        t0, t1 = s.gstart[g], s.gstart[g + 1]
        coff_t[t0:t1] = np.concatenate([[0], np.cumsum(K[t0:t1])[:-1]])
    s.coff_t = coff_t
    s.scol = s.gid * GCOLS + coff_t                     # stream col of tile
    # combined-tensor i16 offsets: per group [idx 512 | mask 512 | ad 2*T_g]
    blk = GCOLS * 16 + 2 * s.gsizes
    s.gco = np.concatenate([[0], np.cumsum(blk)])
    s.CW = int(s.gco[-1])
    s.adoff = s.gco[s.gid] + GCOLS * 16 + 2 * (np.arange(T_TILES) - s.gstart[s.gid])

    # bool per stream col: belongs to a real tile
    used = np.zeros(NG * GCOLS, bool)
    for t in range(T_TILES):
        used[s.scol[t]:s.scol[t] + K[t]] = True
    s.used_cols = used

    cores_data = []
    for c in range(NCORES):
        slot_d, slot_q, mask, deg = per_core[c]
        # new grid: concat of old tiles in perm order; -1 pads (from the old
        # low-degree tail tile) travel with their tile
        grid_old = np.concatenate([orders[c], np.full(SHP - SH, -1, np.int64)])
        grid = grid_old.reshape(T_TILES, P)[perm].reshape(-1)  # [SHP]
        gridpos = np.full(SH, -1, np.int64)
        valid = grid >= 0
        gridpos[grid[valid]] = np.nonzero(valid)[0]
        gp_s = gridpos[slot_d]
        t_s = gp_s // P
        p_s = gp_s % P
        so = np.argsort(gp_s * NROW + slot_q, kind="stable")
        gs_ = gp_s[so]
        rank = np.arange(len(gs_)) - np.searchsorted(gs_, gs_, side="left")
        rank_s = np.empty_like(rank)
        rank_s[so] = rank

        total = NG * GIDX
        idx_arr = np.full(total, -1, np.int64)
        mask_arr = np.full((total, 4), NEG_BIG, np.float32)
        pos = (s.scol[t_s] + rank_s) * P + p_s
        idx_arr[pos] = slot_q
        mask_arr[pos] = mask
        # pads inside real tile cols -> idx 0 (mask stays NEG_BIG)
        iv = idx_arr.reshape(NG * GCOLS, P)
        iv[used] = np.maximum(iv[used], 0)

        comb = np.zeros((P, s.CW), np.int16)
        for g in range(NG):
            co = int(s.gco[g])
            ia = idx_arr[g * GIDX:(g + 1) * GIDX]
            comb[:, co:co + GCOLS * 8] = _wrap_idx(ia.astype(np.int16))
            mt = mask_arr[g * GIDX:(g + 1) * GIDX].reshape(GCOLS, P, 4)
            mt = np.ascontiguousarray(mt.transpose(1, 0, 2)).reshape(P, GCOLS * 4)
            comb[:, co + GCOLS * 8:co + GCOLS * 16] = mt.view(np.int16)
        cores_data.append({"comb": comb, "grid": grid})
    return s, cores_data


def _fill_ad(s, cores_data, ad_full):
    """Write per-layer a_dst values into the combined tensors; returns copies.
    ad_full: [N] fp32 in node-id order."""
    outs = []
    for c in range(NCORES):
        cd = cores_data[c]
        comb = cd["comb"].copy()
        grid = cd["grid"]
        adv = np.zeros(SHP, np.float32)
        valid = grid >= 0
        adv[valid] = ad_full[c * SH + grid[valid]]
        advt = adv.reshape(T_TILES, P)
        for t in range(T_TILES):
            ao = int(s.adoff[t])
            comb[:, ao:ao + 2] = advt[t][:, None].view(np.int16)
        outs.append(comb)
    return outs


def _pack_table(h_bf16_bits, a_s):
    """h_bf16_bits [N, HID] uint16, a_s [N] float32 -> [NROW, ROWW] bf16.
    Row layout (bf16 elems): [h0|h1|h2|h3 (64) | a_s0..3 (4xf32 = 8) | pad]."""
    import ml_dtypes
    tab = np.zeros((NROW, ROWW), np.uint16)
    tab[:, 0:64] = h_bf16_bits.reshape(NROW, 4 * HID)
    tab[:, 64:72] = a_s.astype(np.float32).view(np.uint16).reshape(NROW, 8)
    return tab.view(ml_dtypes.bfloat16)


# ---------------------------------------------------------------------------
# Device programs
# ---------------------------------------------------------------------------

def _build_transform(repeat=1):
    """Launch 1: per core, [h | a_s | a_d] = x_shard @ [W1 | W1u | W1v].
    The attention projections are folded into the weight matrix on the host
    (a_s = h@u = x@(W1 u)), so one PSUM tile per node tile is the whole
    output — no transpose or second matmul.
    Inputs : xt3 [T_TILES, P, F_IN] bf16 (host pre-tiled lhsT layout),
             w1 [KC, P, HID+2] bf16 (augmented)
    Outputs: hasd [SHP, HID+2] fp32 (h | a_s | a_d)
    """
    HA = HID + 2
    nc = bacc.Bacc("TRN2", target_bir_lowering=False, debug=False,
                   num_devices=NCORES)
    xt3 = nc.dram_tensor("xt3", [T_TILES, P, F_IN], BF, kind="ExternalInput").ap()
    w1 = nc.dram_tensor("w1", [KC, P, HA], BF, kind="ExternalInput").ap()
    hasd = nc.dram_tensor("hasd", [SHP, HA], FP, kind="ExternalOutput").ap()
    with TileContext(nc) as tc:
        with tc.tile_pool(name="cst", bufs=1) as cst, \
             tc.tile_pool(name="xk", bufs=4) as xk, \
             tc.tile_pool(name="ob", bufs=2) as obp, \
             tc.tile_pool(name="ps", bufs=2, space="PSUM") as ps:
            w1t = cst.tile([P, KC * HA], BF)
            nc.sync.dma_start(out=w1t[:].rearrange("p (k h) -> p k h", k=KC),
                              in_=w1[:].rearrange("k p h -> p k h"))

            def tbody(t, obufs):
                xtile = xk.tile([P, F_IN], BF, tag="xt", name="xtile")
                nc.sync.dma_start(out=xtile[:], in_=xt3[t])
                psum = ps.tile([P, HA], FP, space="PSUM", tag="ps", name="psum")
                for k in range(KC):
                    nc.tensor.matmul(
                        psum[:],
                        lhsT=xtile[:, k * P:(k + 1) * P],
                        rhs=w1t[:, k * HA:(k + 1) * HA],
                        start=(k == 0), stop=(k == KC - 1))
                gidx = t % GS
                if gidx == 0:
                    obufs[0] = obp.tile([P, GS * HA], FP, tag="obuf",
                                        name="obuf")
                nc.scalar.copy(obufs[0][:, gidx * HA:(gidx + 1) * HA], psum[:])
                if gidx == GS - 1:
                    t0 = t - GS + 1
                    nc.sync.dma_start(
                        out=hasd[t0 * P:(t0 + GS) * P, :]
                            .rearrange("(g p) c -> p g c", p=P),
                        in_=obufs[0][:].rearrange("p (g c) -> p g c", c=HA))

            def body():
                obufs = [None]
                for t in range(T_TILES):
                    tbody(t, obufs)

            if repeat > 1:
                with tc.For_i(0, repeat):
                    body()
            else:
                body()
    nc.compile()
    return nc


def _build_aggregate(s, layer, repeat=1, bench_mode=0, B=5, DEPTH=3,
                     dyn_rep=False):
    """dyn_rep=True adds a 'rep' [1,1] i32 input used as the hardware-loop
    bound, so one loaded executable can be timed at several repeat counts
    (the SWDGE ring assignment NRT picks at load time stays fixed)."""
    """Launches 2 & 3: stream gather + segment softmax + weighted aggregation.

    layer == 1: out = relu(num/den + b1) -> hasd2 [SHP, HID+2] (h' | a_s2 | a_d2)
    layer == 2: out = log_softmax(num/den @ W2 + b2) -> y [SHP, C_OUT]
    Inputs: tab [NROW, ROWW] bf16; comb [128, CW] i16 (idx|mask|a_d per group);
            cnt1 [1, 1] i32 (constant GIDX); vecs [P, HID or C_OUT] (b tiled);
            uv [HID, 2] fp32 (layer 1); w2 [HID, C_OUT] fp32 (layer 2).
    bench_mode: 0 full, 1 gather-only, 2 +softmax, 3 +aggregate (no head),
                10 consumers-only from a constant buffer (no gathers).
    """
    nc = bacc.Bacc("TRN2", target_bir_lowering=False, debug=False,
                   num_devices=NCORES, num_swdge_queues=4)
    NG = s.NG
    K = s.K
    tab = nc.dram_tensor("tab", [NROW, ROWW], BF, kind="ExternalInput").ap()
    comb = nc.dram_tensor("comb", [P, s.CW], I16, kind="ExternalInput").ap()
    cnt1 = nc.dram_tensor("cnt1", [1, 1], I32, kind="ExternalInput").ap()
    if dyn_rep:
        repin = nc.dram_tensor("rep", [1, 1], I32, kind="ExternalInput").ap()
    if layer == 1:
        vecs = nc.dram_tensor("vecs", [P, HID], FP, kind="ExternalInput").ap()
        hasd2 = nc.dram_tensor("hasd2", [SHP, HID], FP,
                               kind="ExternalOutput").ap()
    else:
        vecs = nc.dram_tensor("vecs", [P, C_OUT], FP, kind="ExternalInput").ap()
        w2 = nc.dram_tensor("w2", [HID, C_OUT], FP, kind="ExternalInput").ap()
        y = nc.dram_tensor("y", [SHP, C_OUT], FP, kind="ExternalOutput").ap()

    # B: gather buffers in flight; DEPTH: groups of gather-ahead
    with TileContext(nc) as tc:
        with tc.tile_pool(name="cst", bufs=1) as cst, \
             tc.tile_pool(name="ix", bufs=B) as ixp, \
             tc.tile_pool(name="gr", bufs=B) as grp, \
             tc.tile_pool(name="sc", bufs=3) as scp, \
             tc.tile_pool(name="ou", bufs=3) as oup, \
             tc.tile_pool(name="ob", bufs=2) as obp, \
             tc.tile_pool(name="ps", bufs=2, space="PSUM") as ps:
            vt = cst.tile([P, vecs.shape[1]], FP)
            nc.sync.dma_start(out=vt[:], in_=vecs[:])
            cntt = cst.tile([1, 1], I32)
            nc.sync.dma_start(out=cntt[:], in_=cnt1[:])
            if layer == 2:
                ident = cst.tile([P, P], FP)
                make_identity(nc, ident[:])
                w2t = cst.tile([HID, C_OUT], FP)
                nc.sync.dma_start(out=w2t[:], in_=w2[:])
            nreg = nc.gpsimd.alloc_register("nidx")
            nc.gpsimd.reg_load(nreg, cntt[0:1, 0:1])
            if bench_mode == 10:
                dumA = cst.tile([P, GCOLS * ROWW], BF)
                nc.vector.memset(dumA[:], 0.0)
                dumC = cst.tile([P, GCOLS * 16 + 8], I16)
                nc.vector.memset(dumC[:].bitcast(FP), 0.0)

            def s1_gather(g):
                """Combined DMA + one statically-sized SWDGE gather per group
                (used-column count baked in; no -1 tails are ever scanned)."""
                co = int(s.gco[g])
                tg = int(s.gsizes[g])
                t1 = int(s.gstart[g + 1]) - 1
                ucols = int(s.coff_t[t1] + K[t1])   # used columns in group
                nidx = ucols * P
                gbuf = grp.tile([P, GCOLS * ROWW], BF, tag="grid", name="gbuf")
                cmb = ixp.tile([P, GCOLS * 16 + 2 * tg], I16, tag="cmb",
                               name="cmb")
                nc.sync.dma_start(out=cmb[:],
                                  in_=comb[:, co:co + GCOLS * 16 + 2 * tg])
                _my_dma_gather(
                    nc.gpsimd,
                    gbuf[:, 0:ucols * ROWW].rearrange("p (k w) -> p k w",
                                                      w=ROWW),
                    tab[:, :],
                    cmb[:, 0:ucols * 8],
                    nidx, nidx, ROWW, ROWW, g % 4)
                return {"g": gbuf, "cmb": cmb}

            def s2_softmax(st, t):
                """e = leaky(a_s + a_d) + mask; w = exp(e) (no max-sub)."""
                kt = int(K[t])
                c0 = int(s.coff_t[t])
                gbuf, cmb = st["g"], st["cmb"]
                ao = int(s.adoff[t]) - int(s.gco[s.gid[t]])
                msk_t = cmb[:, GCOLS * 8 + c0 * 8:
                            GCOLS * 8 + (c0 + kt) * 8].bitcast(FP)  # [P,kt*4]
                adcol = cmb[:, ao:ao + 2].bitcast(FP)               # [P, 1]
                g32 = gbuf[:].bitcast(FP)
                as_view = g32.rearrange("p (k u) -> p k u",
                                        u=64)[:, c0:c0 + kt, 32:36]
                lrl = scp.tile([P, kt * 4], FP, tag="lrl", name="lrl")
                nc.scalar.activation(
                    lrl[:].rearrange("p (k u) -> p k u", u=4), as_view,
                    mybir.ActivationFunctionType.Prelu,
                    bias=adcol, scale=1.0, alpha=NEG_SLOPE)
                e = scp.tile([P, kt * 4], FP, tag="e", name="e")
                nc.vector.tensor_tensor(out=e[:], in0=lrl[:], in1=msk_t,
                                        op=mybir.AluOpType.add)
                wts = scp.tile([P, kt * 4], FP, tag="w", name="wts")
                den = scp.tile([P, 1], FP, tag="den", name="den")
                nc.scalar.activation(
                    wts[:], e[:], mybir.ActivationFunctionType.Exp,
                    bias=0.0, scale=1.0, accum_out=den[:])
                st["wts"], st["den"] = wts, den

            def s3_aggregate(st, t):
                """num = sum w*h via halving fold tree; keep num, inv."""
                kt = int(K[t])
                c0 = int(s.coff_t[t])
                gbuf, wts, den = st["g"], st["wts"], st["den"]
                inv = scp.tile([P, 1], FP, tag="inv", name="inv")
                nc.vector.reciprocal(inv[:], den[:])
                h_view = gbuf[:].rearrange("p (k u) -> p k u",
                                           u=ROWW)[:, c0:c0 + kt, 0:4 * HID]
                prod = oup.tile([P, kt * 4 * HID], BF, tag="prod", name="prod")
                nc.vector.tensor_tensor(
                    out=prod[:].rearrange("p (k s w) -> p k s w", s=4, w=HID),
                    in0=h_view.rearrange("p k (s w) -> p k s w", w=HID),
                    in1=wts[:].rearrange("p (k s) -> p k s", s=4)
                        .to_broadcast([P, kt, 4, HID]),
                    op=mybir.AluOpType.mult)
                # halving fold over the combined slot axis m = kt*4
                cur, m, lvl = prod, kt * 4, 0
                while m > 1:
                    half, odd = m // 2, m % 2
                    nm = half + odd
                    dt = FP if nm == 1 else BF
                    nxt = oup.tile([P, nm * HID], dt, tag=f"fh{lvl}",
                                   name="nxt")
                    cv = cur[:].rearrange("p (k w) -> p k w", w=HID)
                    nv = nxt[:].rearrange("p (k w) -> p k w", w=HID)
                    nc.vector.tensor_tensor(
                        out=nv[:, 0:half, :], in0=cv[:, 0:half, :],
                        in1=cv[:, half:2 * half, :], op=mybir.AluOpType.add)
                    if odd:
                        nc.scalar.copy(nv[:, half:half + 1, :],
                                       cv[:, 2 * half:2 * half + 1, :])
                    cur, m, lvl = nxt, nm, lvl + 1
                st["num"], st["inv"] = cur, inv

            def s4_head(st, t, obufs):
                """L1: h' = relu(num/den + b1), scores; L2: z -> log_softmax.
                Both batched GS tiles per output DMA."""
                num, inv = st["num"], st["inv"]
                gidx = t % GS
                if layer == 1:
                    # h' = relu(num/den + b1); layer-2 attention scores are
                    # computed on the host from h' between launches.
                    if gidx == 0:
                        obufs[0] = obp.tile([P, GS * HID], FP,
                                            tag="obuf", name="obuf")
                    ht = obufs[0][:, gidx * HID:(gidx + 1) * HID]
                    nc.vector.scalar_tensor_tensor(
                        out=ht, in0=num[:], scalar=inv[:],
                        in1=vt[:, 0:HID], op0=mybir.AluOpType.mult,
                        op1=mybir.AluOpType.add)
                    nc.vector.tensor_scalar_max(ht, ht, 0.0)
                    if gidx == GS - 1:
                        t0 = t - GS + 1
                        nc.sync.dma_start(
                            out=hasd2[t0 * P:(t0 + GS) * P, :]
                                .rearrange("(g p) c -> p g c", p=P),
                            in_=obufs[0][:].rearrange("p (g c) -> p g c",
                                                      c=HID))
                    return
                pT = ps.tile([HID, P], FP, space="PSUM", tag="pT", name="pT")
                nc.tensor.transpose(pT[:], num[:], ident[:])
                nT = oup.tile([HID, P], FP, tag="nT", name="nT")
                nc.scalar.copy(nT[:], pT[:])
                p2 = ps.tile([P, C_OUT], FP, space="PSUM", tag="p2", name="p2")
                nc.tensor.matmul(p2[:], lhsT=nT[:], rhs=w2t[:],
                                 start=True, stop=True)
                if gidx == 0:
                    obufs[0] = obp.tile([P, GS * C_OUT], FP, tag="obuf",
                                        name="obuf")
                ob = obufs[0]
                # z = (num @ W2)/den + b2
                nc.vector.scalar_tensor_tensor(
                    out=ob[:, gidx * C_OUT:(gidx + 1) * C_OUT], in0=p2[:],
                    scalar=inv[:], in1=vt[:, 0:C_OUT],
                    op0=mybir.AluOpType.mult, op1=mybir.AluOpType.add)
                if gidx != GS - 1:
                    return
                # batched log_softmax over [P, GS, C_OUT] (z bounded, no
                # max-sub needed before exp)
                t0 = t - GS + 1
                ex = oup.tile([P, GS * C_OUT], FP, tag="ex", name="ex")
                nc.scalar.activation(ex[:], ob[:],
                                     mybir.ActivationFunctionType.Exp)
                se = scp.tile([P, GS], FP, tag="se", name="se")
                nc.vector.tensor_reduce(
                    se[:], ex[:].rearrange("p (g c) -> p g c", c=C_OUT),
                    axis=mybir.AxisListType.X, op=mybir.AluOpType.add)
                ls = scp.tile([P, GS], FP, tag="ls", name="ls")
                nc.scalar.activation(ls[:], se[:],
                                     mybir.ActivationFunctionType.Ln)
                nc.vector.tensor_tensor(
                    out=ob[:].rearrange("p (g c) -> p g c", c=C_OUT),
                    in0=ob[:].rearrange("p (g c) -> p g c", c=C_OUT),
                    in1=ls[:].to_broadcast([P, GS, C_OUT]),
                    op=mybir.AluOpType.subtract)
                nc.sync.dma_start(
                    out=y[t0 * P:(t0 + GS) * P, :]
                        .rearrange("(g p) c -> p g c", p=P),
                    in_=ob[:].rearrange("p (g c) -> p g c", c=C_OUT))

            def consume_group(g, obufs):
                for t in range(int(s.gstart[g]), int(s.gstart[g + 1])):
                    st = ({"g": dumA, "cmb": dumC} if bench_mode == 10
                          else stages[g])
                    s2_softmax(st, t)
                    if bench_mode == 2:
                        continue
                    s3_aggregate(st, t)
                    if bench_mode == 3:
                        continue
                    s4_head(st, t, obufs)

            stages = {}

            def body():
                obufs = [None]
                for i in range(NG + DEPTH):
                    if i < NG and bench_mode != 10:
                        stages[i] = s1_gather(i)
                    if bench_mode == 1:
                        continue
                    j = i - DEPTH
                    if 0 <= j < NG:
                        consume_group(j, obufs)
                        if bench_mode != 10:
                            del stages[j]

            if dyn_rep:
                rept = cst.tile([1, 1], I32)
                nc.sync.dma_start(out=rept[:], in_=repin[:])
                rreg = nc.sync.alloc_register("reploop")
                nc.sync.reg_load(rreg, rept[0:1, 0:1])
                with tc.For_i(0, rreg):
                    body()
            elif repeat > 1:
                with tc.For_i(0, repeat):
                    body()
            else:
                body()
    nc.compile()
    return nc


# ---------------------------------------------------------------------------
# Main entry
# ---------------------------------------------------------------------------

LAST_TIMINGS = {}
LAST_STATS = {}
LAST_INPUTS = {}
LAST_SCHED = [None]


def _run_retry(nc, in_maps, cores):
    try:
        return run_bass_kernel_spmd(nc, in_maps, cores)
    except Exception:
        # transient accelerator-unrecoverable states heal on retry
        return run_bass_kernel_spmd(nc, in_maps, cores)


def _prep_transform_inputs(x, W1, att_src1, att_dst1):
    import ml_dtypes

    def to_bf16(a):
        return a.astype(ml_dtypes.bfloat16)

    w1a = np.concatenate([W1, (W1 @ att_src1)[:, None],
                          (W1 @ att_dst1)[:, None]], axis=1)  # [F_IN, HID+2]
    w1r = np.ascontiguousarray(to_bf16(w1a).reshape(KC, P, HID + 2))
    in1 = []
    for c in range(NCORES):
        xsp = np.zeros((SHP, F_IN), np.float32)
        xsp[:SH] = x[c * SH:(c + 1) * SH]
        xt3 = np.ascontiguousarray(
            xsp.reshape(T_TILES, P, KC, P).transpose(0, 3, 2, 1)
            .reshape(T_TILES, P, F_IN))
        in1.append({"xt3": to_bf16(xt3), "w1": w1r})
    return in1


def kernel(x, edge_index, W1, att_src1, att_dst1, b1, W2, att_src2, att_dst2, b2):
    import time as _time
    x = np.asarray(x, np.float32)
    W1 = np.asarray(W1, np.float32)
    W2 = np.asarray(W2, np.float32)
    att_src1 = np.asarray(att_src1, np.float32)
    att_dst1 = np.asarray(att_dst1, np.float32)
    att_src2 = np.asarray(att_src2, np.float32)
    att_dst2 = np.asarray(att_dst2, np.float32)
    b1 = np.asarray(b1, np.float32)
    b2 = np.asarray(b2, np.float32)

    import ml_dtypes

    def bf16_bits(a):
        return a.astype(ml_dtypes.bfloat16).view(np.uint16)

    print("preprocess...", flush=True)
    _t = _time.time()
    s, cores_data = _build_schedule(edge_index)
    LAST_STATS["sumK"] = int(s.K.sum())
    LAST_STATS["NG"] = int(s.NG)
    LAST_STATS["descs_per_core"] = int(s.K.sum()) * P
    LAST_TIMINGS["preprocess"] = _time.time() - _t

    # ---- launch 1: transform -------------------------------------------
    print("build1...", flush=True)
    nc1 = _build_transform()
    in1 = _prep_transform_inputs(x, W1, att_src1, att_dst1)
    _t = _time.time()
    r1 = _run_retry(nc1, in1, list(range(NCORES)))
    LAST_TIMINGS["launch1"] = _time.time() - _t
    print("launch1 done", flush=True)
    hasd1 = np.concatenate(
        [r1.results[c]["hasd"][:SH] for c in range(NCORES)], axis=0)  # [N,18]
    tab1 = _pack_table(bf16_bits(hasd1[:, 0:HID]), hasd1[:, HID])

    cnt1 = np.full((1, 1), GIDX, np.int32)

    # ---- launch 2: layer-1 aggregation ---------------------------------
    print("build2...", flush=True)
    nc2 = _build_aggregate(s, layer=1)
    u2 = W2 @ att_src2
    v2 = W2 @ att_dst2
    vecs1 = np.tile(b1[None, :], (P, 1)).astype(np.float32)
    combs1 = _fill_ad(s, cores_data, hasd1[:, HID + 1].copy())
    in2 = [{"tab": tab1, "comb": combs1[c], "cnt1": cnt1,
            "vecs": vecs1} for c in range(NCORES)]
    _t = _time.time()
    r2 = _run_retry(nc2, in2, list(range(NCORES)))
    LAST_TIMINGS["launch2"] = _time.time() - _t
    print("launch2 done", flush=True)
    h2 = np.empty((N, HID), np.float32)
    for c in range(NCORES):
        grid = cores_data[c]["grid"]
        valid = grid >= 0
        h2[c * SH + grid[valid]] = r2.results[c]["hasd2"][valid]
    a_s2 = h2 @ u2
    a_d2 = h2 @ v2
    tab2 = _pack_table(bf16_bits(h2), a_s2)

    # ---- launch 3: layer-2 aggregation + classifier --------------------
    print("build3...", flush=True)
    nc3 = _build_aggregate(s, layer=2)
    vecs2 = np.tile(b2[None, :], (P, 1)).astype(np.float32)
    combs2 = _fill_ad(s, cores_data, a_d2.astype(np.float32))
    in3 = [{"tab": tab2, "comb": combs2[c], "cnt1": cnt1,
            "vecs": vecs2, "w2": W2} for c in range(NCORES)]
    _t = _time.time()
    r3 = _run_retry(nc3, in3, list(range(NCORES)))
    LAST_TIMINGS["launch3"] = _time.time() - _t
    print("launch3 done", flush=True)

    out = np.zeros((N, C_OUT), np.float32)
    for c in range(NCORES):
        grid = cores_data[c]["grid"]
        valid = grid >= 0
        out[c * SH + grid[valid]] = r3.results[c]["y"][valid]
    LAST_INPUTS.update({"in1": in1, "in2": in2, "in3": in3})
    LAST_SCHED[0] = s
    return out


# revision 31
# speedup vs baseline: 1.2043x; 1.2043x over previous
"""GAT (2-layer, PyG-style GATConv) on 8 Trainium2 NeuronCores. v2

Strategy (dst-sharded, quad-packed stream gather):
- Nodes sharded by dst across 8 cores (12500 each); edges partitioned by dst
  core; segment-softmax + weighted aggregation local per dst shard.
- Node table packs FOUR nodes per 256B row ([4 x 16 bf16 h | 4 x fp32 a_src |
  pad]); a per-slot additive mask (0 / ln(mult) / -1e30) selects the sub-row
  inside the segment softmax (also handling duplicate edges exactly); exp()
  runs without max-subtraction (|e| <~ 30, safe in fp32).
- Dst tiles are degree-sorted then FFD bin-packed into GROUPS of <= 64
  columns (8192 idxs) and relabeled in group order. Each group is ONE
  fixed-size SWDGE gather instruction (constant idx-count register, -1 tail
  idxs skipped by ucode) -> the Pool engine runs a clean stream of maximal
  gathers with zero per-tile register traffic (measured ~2.1 ns/idx vs 4.56
  in the per-tile variant).
- Per-tile epilogue: Prelu(a_s + a_d) + mask -> exp (ACT, accum gives the
  denominator); weighted h sum via a halving fold tree (contiguous DVE adds
  only; the old strided tensor_reduce cost ~10x per element).
- 3 SPMD launches: transform (x@W1 + scores, host pre-tiled so DMA lines are
  1KB contiguous) / layer-1 aggregation / layer-2 aggregation + classifier +
  log_softmax. The gather schedule is identical for both layers.
"""

import numpy as np

import concourse.ap_utils as ap_utils
import concourse.bacc as bacc
import concourse.bass as bass
import concourse.mybir as mybir
from concourse.bass import round_up_to_multiple
from concourse.bass_utils import run_bass_kernel_spmd
from concourse.masks import make_identity
from concourse.tile import TileContext

P = 128
NCORES = 8
N = 100000
F_IN = 512
HID = 16
C_OUT = 32
NEG_SLOPE = 0.2
NROW = N // 4          # 25000 quad rows
ROWW = 128             # bf16 elems per table row (256B)
SH = N // NCORES       # nodes per core
T_TILES = (SH + P - 1) // P
SHP = T_TILES * P      # padded shard size (12544)
KC = F_IN // P
NEG_BIG = -1.0e30
GCOLS = 64             # columns per gather group (8192 idxs)
GIDX = GCOLS * P
GS = 7                 # tiles per output batch

FP = mybir.dt.float32
BF = mybir.dt.bfloat16
I16 = mybir.dt.int16
I32 = mybir.dt.int32


def _my_dma_gather(gp, out_ap, in_ap, idxs_ap, num_idxs, num_idxs_reg,
                   elem_size, elem_step, queue_num):
    """BassGpSimd.dma_gather (non-transpose, DRAM source) without the
    256B-elem_size restriction and with a runtime num_idxs register; the row
    stride (elem_step) must still be a multiple of 256B."""
    assert idxs_ap.dtype == I16
    assert in_ap.dtype == out_ap.dtype
    assert in_ap.space == bass.MemorySpace.DRAM
    assert idxs_ap.space == bass.MemorySpace.SBUF
    assert out_ap.space == bass.MemorySpace.SBUF
    assert ap_utils.ap_is_contiguous(out_ap.ap[1:])
    assert ap_utils.ap_is_contiguous(idxs_ap.ap[1:])
    assert in_ap.ap[-1][1] == out_ap.ap[-1][1] == elem_size
    assert out_ap.ap[0][1] * out_ap.ap[1][1] == round_up_to_multiple(num_idxs, 128)
    assert in_ap.ap[0][0] == elem_step
    stride_bytes = elem_step * mybir.dt.size(in_ap.dtype)
    assert stride_bytes % 256 == 0 and stride_bytes // 256 < 256
    _in_ap = gp.lower_ap_dma(in_ap, for_custom_bir_dma=True)
    _idxs_ap = gp.lower_ap(idxs_ap)
    _out_ap = gp.lower_ap(out_ap)
    return gp.add_instruction(
        mybir.InstDMAGatherAnt(
            name=gp.bass.get_next_instruction_name(),
            ins=[*_in_ap, _idxs_ap, gp.lower_val_access(gp.to_reg(num_idxs_reg))],
            outs=[_out_ap],
            transpose=False,
            num_idxs=num_idxs,
            elem_size=elem_size,
            stride_bytes_256=stride_bytes // 256,
            gen_mode=0,
            single_packet=False,
            queue_num=queue_num,
        )
    )


# ---------------------------------------------------------------------------
# Host-side preprocessing
# ---------------------------------------------------------------------------

def _wrap_idx(local_idx):
    """Wrap an int16 index list [M] (M % 128 == 0) into the SWDGE layout
    [128, M//16]: idx i at partition i%16, col i//16, replicated x8."""
    M = local_idx.shape[0]
    w = local_idx.reshape(M // 16, 16).T.astype(np.int16)  # [16, M//16]
    return np.tile(w, (8, 1))


class _Sched:
    pass


def _build_schedule(edge_index):
    """Shared (both layers) gather schedule.

    Tiles are degree-sorted, FFD bin-packed into groups of <= GCOLS columns,
    then relabeled so group members are consecutive. Per core, one combined
    int16 tensor [128, CW] holds per group: [idx GCOLS*8 | mask GCOLS*8 |
    a_dst 2*T_g] (a_dst filled per layer later).
    """
    src = np.asarray(edge_index[0], dtype=np.int64)
    dst = np.asarray(edge_index[1], dtype=np.int64)
    loops = np.arange(N, dtype=np.int64)
    src = np.concatenate([src, loops])
    dst = np.concatenate([dst, loops])
    core = dst // SH

    per_core = []
    for c in range(NCORES):
        m = core == c
        s_c = src[m]
        d_loc = dst[m] - c * SH
        quad = s_c >> 2
        sub = (s_c & 3).astype(np.int64)
        # slots: unique (d_loc, quad); per-(slot, sub) multiplicity
        ekey = (d_loc * NROW + quad) * 4 + sub
        uk, ucnt = np.unique(ekey, return_counts=True)
        skey = uk >> 2
        usub = (uk & 3).astype(np.int64)
        slot_ids, slot_inv = np.unique(skey, return_inverse=True)
        nslots = len(slot_ids)
        slot_d = slot_ids // NROW
        slot_q = (slot_ids % NROW).astype(np.int64)
        mask = np.full((nslots, 4), NEG_BIG, np.float32)
        mask[slot_inv, usub] = np.log(ucnt).astype(np.float32)
        deg = np.bincount(slot_d, minlength=SH)
        per_core.append((slot_d, slot_q, mask, deg))

    # shared degree-sorted tiling; K[t] = max over cores
    orders = [np.argsort(-pc[3], kind="stable").astype(np.int64)
              for pc in per_core]
    Kc = np.zeros((NCORES, T_TILES), np.int64)
    for c in range(NCORES):
        deg = per_core[c][3]
        ds = deg[orders[c]]
        grid = np.concatenate([ds, np.zeros(SHP - SH, np.int64)]).reshape(T_TILES, P)
        Kc[c] = grid.max(axis=1)
    K = np.maximum(Kc.max(axis=0), 1)
    assert int(K.max()) <= GCOLS

    # FFD bin packing of tiles into groups of <= GCOLS columns
    order_t = np.argsort(-K, kind="stable")
    bins = []  # [used_cols, [old_tile,...]]
    for t in order_t:
        kt = int(K[t])
        for b in bins:
            if b[0] + kt <= GCOLS:
                b[0] += kt
                b[1].append(int(t))
                break
        else:
            bins.append([kt, [int(t)]])
    perm = np.array([t for b in bins for t in b[1]], np.int64)  # new -> old
    K = K[perm]
    gsizes = [len(b[1]) for b in bins]
    NG = len(bins)

    s = _Sched()
    s.K = K
    s.NG = NG
    s.gsizes = np.array(gsizes, np.int64)
    s.gid = np.repeat(np.arange(NG), s.gsizes)          # tile -> group
    s.gstart = np.concatenate([[0], np.cumsum(s.gsizes)])  # first tile of grp
    # column offset of each tile inside its group
    coff_t = np.zeros(T_TILES, np.int64)
    for g in range(NG):
        t0, t1 = s.gstart[g], s.gstart[g + 1]
        coff_t[t0:t1] = np.concatenate([[0], np.cumsum(K[t0:t1])[:-1]])
    s.coff_t = coff_t
    s.scol = s.gid * GCOLS + coff_t                     # stream col of tile
    # combined-tensor i16 offsets: per group [idx 512 | mask 512 | ad 2*T_g]
    blk = GCOLS * 16 + 2 * s.gsizes
    s.gco = np.concatenate([[0], np.cumsum(blk)])
    s.CW = int(s.gco[-1])
    s.adoff = s.gco[s.gid] + GCOLS * 16 + 2 * (np.arange(T_TILES) - s.gstart[s.gid])

    # bool per stream col: belongs to a real tile
    used = np.zeros(NG * GCOLS, bool)
    for t in range(T_TILES):
        used[s.scol[t]:s.scol[t] + K[t]] = True
    s.used_cols = used

    cores_data = []
    for c in range(NCORES):
        slot_d, slot_q, mask, deg = per_core[c]
        # new grid: concat of old tiles in perm order; -1 pads (from the old
        # low-degree tail tile) travel with their tile
        grid_old = np.concatenate([orders[c], np.full(SHP - SH, -1, np.int64)])
        grid = grid_old.reshape(T_TILES, P)[perm].reshape(-1)  # [SHP]
        gridpos = np.full(SH, -1, np.int64)
        valid = grid >= 0
        gridpos[grid[valid]] = np.nonzero(valid)[0]
        gp_s = gridpos[slot_d]
        t_s = gp_s // P
        p_s = gp_s % P
        so = np.argsort(gp_s * NROW + slot_q, kind="stable")
        gs_ = gp_s[so]
        rank = np.arange(len(gs_)) - np.searchsorted(gs_, gs_, side="left")
        rank_s = np.empty_like(rank)
        rank_s[so] = rank

        total = NG * GIDX
        idx_arr = np.full(total, -1, np.int64)
        mask_arr = np.full((total, 4), NEG_BIG, np.float32)
        pos = (s.scol[t_s] + rank_s) * P + p_s
        idx_arr[pos] = slot_q
        mask_arr[pos] = mask
        # pads inside real tile cols -> idx 0 (mask stays NEG_BIG)
        iv = idx_arr.reshape(NG * GCOLS, P)
        iv[used] = np.maximum(iv[used], 0)

        comb = np.zeros((P, s.CW), np.int16)
        for g in range(NG):
            co = int(s.gco[g])
            ia = idx_arr[g * GIDX:(g + 1) * GIDX]
            comb[:, co:co + GCOLS * 8] = _wrap_idx(ia.astype(np.int16))
            mt = mask_arr[g * GIDX:(g + 1) * GIDX].reshape(GCOLS, P, 4)
            mt = np.ascontiguousarray(mt.transpose(1, 0, 2)).reshape(P, GCOLS * 4)
            comb[:, co + GCOLS * 8:co + GCOLS * 16] = mt.view(np.int16)
        cores_data.append({"comb": comb, "grid": grid})
    return s, cores_data


def _fill_ad(s, cores_data, ad_full):
    """Write per-layer a_dst values into the combined tensors; returns copies.
    ad_full: [N] fp32 in node-id order."""
    outs = []
    for c in range(NCORES):
        cd = cores_data[c]
        comb = cd["comb"].copy()
        grid = cd["grid"]
        adv = np.zeros(SHP, np.float32)
        valid = grid >= 0
        adv[valid] = ad_full[c * SH + grid[valid]]
        advt = adv.reshape(T_TILES, P)
        for t in range(T_TILES):
            ao = int(s.adoff[t])
            comb[:, ao:ao + 2] = advt[t][:, None].view(np.int16)
        outs.append(comb)
    return outs


def _pack_table(h_bf16_bits, a_s):
    """h_bf16_bits [N, HID] uint16, a_s [N] float32 -> [NROW, ROWW] bf16.
    Row layout (bf16 elems): [h0|h1|h2|h3 (64) | a_s0..3 (4xf32 = 8) | pad]."""
    import ml_dtypes
    tab = np.zeros((NROW, ROWW), np.uint16)
    tab[:, 0:64] = h_bf16_bits.reshape(NROW, 4 * HID)
    tab[:, 64:72] = a_s.astype(np.float32).view(np.uint16).reshape(NROW, 8)
    return tab.view(ml_dtypes.bfloat16)


# ---------------------------------------------------------------------------
# Device programs
# ---------------------------------------------------------------------------

def _build_transform(repeat=1):
    """Launch 1: per core, [h | a_s | a_d] = x_shard @ [W1 | W1u | W1v].
    The attention projections are folded into the weight matrix on the host
    (a_s = h@u = x@(W1 u)), so one PSUM tile per node tile is the whole
    output — no transpose or second matmul.
    Inputs : xt3 [T_TILES, P, F_IN] bf16 (host pre-tiled lhsT layout),
             w1 [KC, P, HID+2] bf16 (augmented)
    Outputs: hasd [SHP, HID+2] fp32 (h | a_s | a_d)
    """
    HA = HID + 2
    nc = bacc.Bacc("TRN2", target_bir_lowering=False, debug=False,
                   num_devices=NCORES)
    xt3 = nc.dram_tensor("xt3", [T_TILES, P, F_IN], BF, kind="ExternalInput").ap()
    w1 = nc.dram_tensor("w1", [KC, P, HA], BF, kind="ExternalInput").ap()
    hasd = nc.dram_tensor("hasd", [SHP, HA], FP, kind="ExternalOutput").ap()
    with TileContext(nc) as tc:
        with tc.tile_pool(name="cst", bufs=1) as cst, \
             tc.tile_pool(name="xk", bufs=4) as xk, \
             tc.tile_pool(name="ob", bufs=2) as obp, \
             tc.tile_pool(name="ps", bufs=2, space="PSUM") as ps:
            w1t = cst.tile([P, KC * HA], BF)
            nc.sync.dma_start(out=w1t[:].rearrange("p (k h) -> p k h", k=KC),
                              in_=w1[:].rearrange("k p h -> p k h"))

            def tbody(t, obufs):
                xtile = xk.tile([P, F_IN], BF, tag="xt", name="xtile")
                nc.sync.dma_start(out=xtile[:], in_=xt3[t])
                psum = ps.tile([P, HA], FP, space="PSUM", tag="ps", name="psum")
                for k in range(KC):
                    nc.tensor.matmul(
                        psum[:],
                        lhsT=xtile[:, k * P:(k + 1) * P],
                        rhs=w1t[:, k * HA:(k + 1) * HA],
                        start=(k == 0), stop=(k == KC - 1))
                gidx = t % GS
                if gidx == 0:
                    obufs[0] = obp.tile([P, GS * HA], FP, tag="obuf",
                                        name="obuf")
                nc.scalar.copy(obufs[0][:, gidx * HA:(gidx + 1) * HA], psum[:])
                if gidx == GS - 1:
                    t0 = t - GS + 1
                    nc.sync.dma_start(
                        out=hasd[t0 * P:(t0 + GS) * P, :]
                            .rearrange("(g p) c -> p g c", p=P),
                        in_=obufs[0][:].rearrange("p (g c) -> p g c", c=HA))

            def body():
                obufs = [None]
                for t in range(T_TILES):
                    tbody(t, obufs)

            if repeat > 1:
                with tc.For_i(0, repeat):
                    body()
            else:
                body()
    nc.compile()
    return nc


def _build_aggregate(s, layer, repeat=1, bench_mode=0, B=5, DEPTH=3,
                     dyn_rep=False, sbuf_mimic=False):
    """dyn_rep=True adds a 'rep' [1,1] i32 input used as the hardware-loop
    bound, so one loaded executable can be timed at several repeat counts
    (the SWDGE ring assignment NRT picks at load time stays fixed)."""
    """Launches 2 & 3: stream gather + segment softmax + weighted aggregation.

    layer == 1: out = relu(num/den + b1) -> hasd2 [SHP, HID+2] (h' | a_s2 | a_d2)
    layer == 2: out = log_softmax(num/den @ W2 + b2) -> y [SHP, C_OUT]
    Inputs: tab [NROW, ROWW] bf16; comb [128, CW] i16 (idx|mask|a_d per group);
            cnt1 [1, 1] i32 (constant GIDX); vecs [P, HID or C_OUT] (b tiled);
            uv [HID, 2] fp32 (layer 1); w2 [HID, C_OUT] fp32 (layer 2).
    bench_mode: 0 full, 1 gather-only, 2 +softmax, 3 +aggregate (no head),
                10 consumers-only from a constant buffer (no gathers).
    """
    nc = bacc.Bacc("TRN2", target_bir_lowering=False, debug=False,
                   num_devices=NCORES, num_swdge_queues=4)
    NG = s.NG
    K = s.K
    tab = nc.dram_tensor("tab", [NROW, ROWW], BF, kind="ExternalInput").ap()
    comb = nc.dram_tensor("comb", [P, s.CW], I16, kind="ExternalInput").ap()
    cnt1 = nc.dram_tensor("cnt1", [1, 1], I32, kind="ExternalInput").ap()
    if dyn_rep:
        repin = nc.dram_tensor("rep", [1, 1], I32, kind="ExternalInput").ap()
    # Both layers declare identically-shaped buffers so the per-call jax/BFC
    # upload sequence (and hence tab's physical HBM placement, which the
    # random gather is sensitive to) is the same for both programs. Layer 1
    # uses only vecs[:, :HID] and cols [g*C_OUT, g*C_OUT+HID) of the output.
    vecs = nc.dram_tensor("vecs", [P, C_OUT], FP, kind="ExternalInput").ap()
    w2 = nc.dram_tensor("w2", [HID, C_OUT], FP, kind="ExternalInput").ap()
    if layer == 1:
        hasd2 = nc.dram_tensor("hasd2", [SHP, C_OUT], FP,
                               kind="ExternalOutput").ap()
    else:
        y = nc.dram_tensor("y", [SHP, C_OUT], FP, kind="ExternalOutput").ap()

    # B: gather buffers in flight; DEPTH: groups of gather-ahead
    with TileContext(nc) as tc:
        with tc.tile_pool(name="cst", bufs=1) as cst, \
             tc.tile_pool(name="ix", bufs=B) as ixp, \
             tc.tile_pool(name="gr", bufs=B) as grp, \
             tc.tile_pool(name="sc", bufs=3) as scp, \
             tc.tile_pool(name="ou", bufs=3) as oup, \
             tc.tile_pool(name="ob", bufs=2) as obp, \
             tc.tile_pool(name="ps", bufs=2, space="PSUM") as ps:
            vt = cst.tile([P, vecs.shape[1]], FP)
            nc.sync.dma_start(out=vt[:], in_=vecs[:])
            cntt = cst.tile([1, 1], I32)
            nc.sync.dma_start(out=cntt[:], in_=cnt1[:])
            ident = cst.tile([P, P], FP)
            make_identity(nc, ident[:])
            w2t = cst.tile([HID, C_OUT], FP)
            nc.sync.dma_start(out=w2t[:], in_=w2[:])
            if layer == 1 and sbuf_mimic:
                # reserve the pool slots layer 2 uses so both programs get
                # identical SBUF maps (testing placement-sensitivity of the
                # gather rate)
                dnT = oup.tile([HID, P], FP, tag="nT", name="dnT")
                nc.vector.memset(dnT[:], 0.0)
                dex = oup.tile([P, GS * C_OUT], FP, tag="ex", name="dex")
                nc.vector.memset(dex[:], 0.0)
                dse = scp.tile([P, GS], FP, tag="se", name="dse")
                nc.vector.memset(dse[:], 0.0)
                dls = scp.tile([P, GS], FP, tag="ls", name="dls")
                nc.vector.memset(dls[:], 0.0)
                dpT = ps.tile([HID, P], FP, space="PSUM", tag="pT", name="dpT")
                dp2 = ps.tile([P, C_OUT], FP, space="PSUM", tag="p2",
                              name="dp2")
                del dpT, dp2
            nreg = nc.gpsimd.alloc_register("nidx")
            nc.gpsimd.reg_load(nreg, cntt[0:1, 0:1])
            if bench_mode == 10:
                dumA = cst.tile([P, GCOLS * ROWW], BF)
                nc.vector.memset(dumA[:], 0.0)
                dumC = cst.tile([P, GCOLS * 16 + 8], I16)
                nc.vector.memset(dumC[:].bitcast(FP), 0.0)

            def s1_gather(g):
                """Combined DMA + one statically-sized SWDGE gather per group
                (used-column count baked in; no -1 tails are ever scanned)."""
                co = int(s.gco[g])
                tg = int(s.gsizes[g])
                t1 = int(s.gstart[g + 1]) - 1
                ucols = int(s.coff_t[t1] + K[t1])   # used columns in group
                nidx = ucols * P
                gbuf = grp.tile([P, GCOLS * ROWW], BF, tag="grid", name="gbuf")
                cmb = ixp.tile([P, GCOLS * 16 + 2 * tg], I16, tag="cmb",
                               name="cmb")
                nc.sync.dma_start(out=cmb[:],
                                  in_=comb[:, co:co + GCOLS * 16 + 2 * tg])
                _my_dma_gather(
                    nc.gpsimd,
                    gbuf[:, 0:ucols * ROWW].rearrange("p (k w) -> p k w",
                                                      w=ROWW),
                    tab[:, :],
                    cmb[:, 0:ucols * 8],
                    nidx, nidx, ROWW, ROWW, g % 4)
                return {"g": gbuf, "cmb": cmb}

            def s2_softmax(st, t):
                """e = leaky(a_s + a_d) + mask; w = exp(e) (no max-sub)."""
                kt = int(K[t])
                c0 = int(s.coff_t[t])
                gbuf, cmb = st["g"], st["cmb"]
                ao = int(s.adoff[t]) - int(s.gco[s.gid[t]])
                msk_t = cmb[:, GCOLS * 8 + c0 * 8:
                            GCOLS * 8 + (c0 + kt) * 8].bitcast(FP)  # [P,kt*4]
                adcol = cmb[:, ao:ao + 2].bitcast(FP)               # [P, 1]
                g32 = gbuf[:].bitcast(FP)
                as_view = g32.rearrange("p (k u) -> p k u",
                                        u=64)[:, c0:c0 + kt, 32:36]
                lrl = scp.tile([P, kt * 4], FP, tag="lrl", name="lrl")
                nc.scalar.activation(
                    lrl[:].rearrange("p (k u) -> p k u", u=4), as_view,
                    mybir.ActivationFunctionType.Prelu,
                    bias=adcol, scale=1.0, alpha=NEG_SLOPE)
                e = scp.tile([P, kt * 4], FP, tag="e", name="e")
                nc.vector.tensor_tensor(out=e[:], in0=lrl[:], in1=msk_t,
                                        op=mybir.AluOpType.add)
                wts = scp.tile([P, kt * 4], FP, tag="w", name="wts")
                den = scp.tile([P, 1], FP, tag="den", name="den")
                nc.scalar.activation(
                    wts[:], e[:], mybir.ActivationFunctionType.Exp,
                    bias=0.0, scale=1.0, accum_out=den[:])
                st["wts"], st["den"] = wts, den

            def s3_aggregate(st, t):
                """num = sum w*h via halving fold tree; keep num, inv."""
                kt = int(K[t])
                c0 = int(s.coff_t[t])
                gbuf, wts, den = st["g"], st["wts"], st["den"]
                inv = scp.tile([P, 1], FP, tag="inv", name="inv")
                nc.vector.reciprocal(inv[:], den[:])
                h_view = gbuf[:].rearrange("p (k u) -> p k u",
                                           u=ROWW)[:, c0:c0 + kt, 0:4 * HID]
                prod = oup.tile([P, kt * 4 * HID], BF, tag="prod", name="prod")
                nc.vector.tensor_tensor(
                    out=prod[:].rearrange("p (k s w) -> p k s w", s=4, w=HID),
                    in0=h_view.rearrange("p k (s w) -> p k s w", w=HID),
                    in1=wts[:].rearrange("p (k s) -> p k s", s=4)
                        .to_broadcast([P, kt, 4, HID]),
                    op=mybir.AluOpType.mult)
                # halving fold over the combined slot axis m = kt*4
                cur, m, lvl = prod, kt * 4, 0
                while m > 1:
                    half, odd = m // 2, m % 2
                    nm = half + odd
                    dt = FP if nm == 1 else BF
                    nxt = oup.tile([P, nm * HID], dt, tag=f"fh{lvl}",
                                   name="nxt")
                    cv = cur[:].rearrange("p (k w) -> p k w", w=HID)
                    nv = nxt[:].rearrange("p (k w) -> p k w", w=HID)
                    nc.vector.tensor_tensor(
                        out=nv[:, 0:half, :], in0=cv[:, 0:half, :],
                        in1=cv[:, half:2 * half, :], op=mybir.AluOpType.add)
                    if odd:
                        nc.scalar.copy(nv[:, half:half + 1, :],
                                       cv[:, 2 * half:2 * half + 1, :])
                    cur, m, lvl = nxt, nm, lvl + 1
                st["num"], st["inv"] = cur, inv

            def s4_head(st, t, obufs):
                """L1: h' = relu(num/den + b1), scores; L2: z -> log_softmax.
                Both batched GS tiles per output DMA."""
                num, inv = st["num"], st["inv"]
                gidx = t % GS
                if layer == 1:
                    # h' = relu(num/den + b1); layer-2 attention scores are
                    # computed on the host from h' between launches. Output
                    # rows padded to C_OUT (only [:HID] meaningful).
                    if gidx == 0:
                        obufs[0] = obp.tile([P, GS * C_OUT], FP,
                                            tag="obuf", name="obuf")
                        nc.vector.memset(obufs[0][:], 0.0)
                    ht = obufs[0][:, gidx * C_OUT:gidx * C_OUT + HID]
                    # stt on DVE into scratch; relu out-of-place on ACT (an
                    # in-place DVE max here co-occurred with a 1.5x slower
                    # gather stream)
                    tmp = oup.tile([P, HID], FP, tag="htmp", name="htmp")
                    nc.vector.scalar_tensor_tensor(
                        out=tmp[:], in0=num[:], scalar=inv[:],
                        in1=vt[:, 0:HID], op0=mybir.AluOpType.mult,
                        op1=mybir.AluOpType.add)
                    nc.scalar.activation(ht, tmp[:],
                                         mybir.ActivationFunctionType.Relu)
                    if gidx == GS - 1:
                        t0 = t - GS + 1
                        nc.sync.dma_start(
                            out=hasd2[t0 * P:(t0 + GS) * P, :]
                                .rearrange("(g p) c -> p g c", p=P),
                            in_=obufs[0][:].rearrange("p (g c) -> p g c",
                                                      c=C_OUT))
                    return
                pT = ps.tile([HID, P], FP, space="PSUM", tag="pT", name="pT")
                nc.tensor.transpose(pT[:], num[:], ident[:])
                nT = oup.tile([HID, P], FP, tag="nT", name="nT")
                nc.scalar.copy(nT[:], pT[:])
                p2 = ps.tile([P, C_OUT], FP, space="PSUM", tag="p2", name="p2")
                nc.tensor.matmul(p2[:], lhsT=nT[:], rhs=w2t[:],
                                 start=True, stop=True)
                if gidx == 0:
                    obufs[0] = obp.tile([P, GS * C_OUT], FP, tag="obuf",
                                        name="obuf")
                ob = obufs[0]
                # z = (num @ W2)/den + b2
                nc.vector.scalar_tensor_tensor(
                    out=ob[:, gidx * C_OUT:(gidx + 1) * C_OUT], in0=p2[:],
                    scalar=inv[:], in1=vt[:, 0:C_OUT],
                    op0=mybir.AluOpType.mult, op1=mybir.AluOpType.add)
                if gidx != GS - 1:
                    return
                # batched log_softmax over [P, GS, C_OUT] (z bounded, no
                # max-sub needed before exp)
                t0 = t - GS + 1
                ex = oup.tile([P, GS * C_OUT], FP, tag="ex", name="ex")
                nc.scalar.activation(ex[:], ob[:],
                                     mybir.ActivationFunctionType.Exp)
                se = scp.tile([P, GS], FP, tag="se", name="se")
                nc.vector.tensor_reduce(
                    se[:], ex[:].rearrange("p (g c) -> p g c", c=C_OUT),
                    axis=mybir.AxisListType.X, op=mybir.AluOpType.add)
                ls = scp.tile([P, GS], FP, tag="ls", name="ls")
                nc.scalar.activation(ls[:], se[:],
                                     mybir.ActivationFunctionType.Ln)
                nc.vector.tensor_tensor(
                    out=ob[:].rearrange("p (g c) -> p g c", c=C_OUT),
                    in0=ob[:].rearrange("p (g c) -> p g c", c=C_OUT),
                    in1=ls[:].to_broadcast([P, GS, C_OUT]),
                    op=mybir.AluOpType.subtract)
                nc.sync.dma_start(
                    out=y[t0 * P:(t0 + GS) * P, :]
                        .rearrange("(g p) c -> p g c", p=P),
                    in_=ob[:].rearrange("p (g c) -> p g c", c=C_OUT))

            def consume_group(g, obufs):
                for t in range(int(s.gstart[g]), int(s.gstart[g + 1])):
                    st = ({"g": dumA, "cmb": dumC} if bench_mode == 10
                          else stages[g])
                    s2_softmax(st, t)
                    if bench_mode == 2:
                        continue
                    s3_aggregate(st, t)
                    if bench_mode == 3:
                        continue
                    s4_head(st, t, obufs)

            stages = {}

            def body():
                obufs = [None]
                for i in range(NG + DEPTH):
                    if i < NG and bench_mode != 10:
                        stages[i] = s1_gather(i)
                    if bench_mode == 1:
                        continue
                    j = i - DEPTH
                    if 0 <= j < NG:
                        consume_group(j, obufs)
                        if bench_mode != 10:
                            del stages[j]

            if dyn_rep:
                rept = cst.tile([1, 1], I32)
                nc.sync.dma_start(out=rept[:], in_=repin[:])
                rreg = nc.sync.alloc_register("reploop")
                nc.sync.reg_load(rreg, rept[0:1, 0:1])
                with tc.For_i(0, rreg):
                    body()
            elif repeat > 1:
                with tc.For_i(0, repeat):
                    body()
            else:
                body()
    nc.compile()
    return nc


# ---------------------------------------------------------------------------
# Main entry
# ---------------------------------------------------------------------------

LAST_TIMINGS = {}
LAST_STATS = {}
LAST_INPUTS = {}
LAST_SCHED = [None]


def _run_retry(nc, in_maps, cores):
    try:
        return run_bass_kernel_spmd(nc, in_maps, cores)
    except Exception:
        # transient accelerator-unrecoverable states heal on retry
        return run_bass_kernel_spmd(nc, in_maps, cores)


def _prep_transform_inputs(x, W1, att_src1, att_dst1):
    import ml_dtypes

    def to_bf16(a):
        return a.astype(ml_dtypes.bfloat16)

    w1a = np.concatenate([W1, (W1 @ att_src1)[:, None],
                          (W1 @ att_dst1)[:, None]], axis=1)  # [F_IN, HID+2]
    w1r = np.ascontiguousarray(to_bf16(w1a).reshape(KC, P, HID + 2))
    in1 = []
    for c in range(NCORES):
        xsp = np.zeros((SHP, F_IN), np.float32)
        xsp[:SH] = x[c * SH:(c + 1) * SH]
        xt3 = np.ascontiguousarray(
            xsp.reshape(T_TILES, P, KC, P).transpose(0, 3, 2, 1)
            .reshape(T_TILES, P, F_IN))
        in1.append({"xt3": to_bf16(xt3), "w1": w1r})
    return in1


def kernel(x, edge_index, W1, att_src1, att_dst1, b1, W2, att_src2, att_dst2, b2):
    import time as _time
    x = np.asarray(x, np.float32)
    W1 = np.asarray(W1, np.float32)
    W2 = np.asarray(W2, np.float32)
    att_src1 = np.asarray(att_src1, np.float32)
    att_dst1 = np.asarray(att_dst1, np.float32)
    att_src2 = np.asarray(att_src2, np.float32)
    att_dst2 = np.asarray(att_dst2, np.float32)
    b1 = np.asarray(b1, np.float32)
    b2 = np.asarray(b2, np.float32)

    import ml_dtypes

    def bf16_bits(a):
        return a.astype(ml_dtypes.bfloat16).view(np.uint16)

    print("preprocess...", flush=True)
    _t = _time.time()
    s, cores_data = _build_schedule(edge_index)
    LAST_STATS["sumK"] = int(s.K.sum())
    LAST_STATS["NG"] = int(s.NG)
    LAST_STATS["descs_per_core"] = int(s.K.sum()) * P
    LAST_TIMINGS["preprocess"] = _time.time() - _t

    # ---- launch 1: transform -------------------------------------------
    print("build1...", flush=True)
    nc1 = _build_transform()
    in1 = _prep_transform_inputs(x, W1, att_src1, att_dst1)
    _t = _time.time()
    r1 = _run_retry(nc1, in1, list(range(NCORES)))
    LAST_TIMINGS["launch1"] = _time.time() - _t
    print("launch1 done", flush=True)
    hasd1 = np.concatenate(
        [r1.results[c]["hasd"][:SH] for c in range(NCORES)], axis=0)  # [N,18]
    tab1 = _pack_table(bf16_bits(hasd1[:, 0:HID]), hasd1[:, HID])

    cnt1 = np.full((1, 1), GIDX, np.int32)

    # ---- launch 2: layer-1 aggregation ---------------------------------
    print("build2...", flush=True)
    nc2 = _build_aggregate(s, layer=1)
    u2 = W2 @ att_src2
    v2 = W2 @ att_dst2
    vecs1 = np.zeros((P, C_OUT), np.float32)
    vecs1[:, 0:HID] = b1[None, :]
    combs1 = _fill_ad(s, cores_data, hasd1[:, HID + 1].copy())
    in2 = [{"tab": tab1, "comb": combs1[c], "cnt1": cnt1,
            "vecs": vecs1, "w2": W2} for c in range(NCORES)]
    _t = _time.time()
    r2 = _run_retry(nc2, in2, list(range(NCORES)))
    LAST_TIMINGS["launch2"] = _time.time() - _t
    print("launch2 done", flush=True)
    h2 = np.empty((N, HID), np.float32)
    for c in range(NCORES):
        grid = cores_data[c]["grid"]
        valid = grid >= 0
        h2[c * SH + grid[valid]] = r2.results[c]["hasd2"][valid][:, 0:HID]
    a_s2 = h2 @ u2
    a_d2 = h2 @ v2
    tab2 = _pack_table(bf16_bits(h2), a_s2)

    # ---- launch 3: layer-2 aggregation + classifier --------------------
    print("build3...", flush=True)
    nc3 = _build_aggregate(s, layer=2)
    vecs2 = np.tile(b2[None, :], (P, 1)).astype(np.float32)
    combs2 = _fill_ad(s, cores_data, a_d2.astype(np.float32))
    in3 = [{"tab": tab2, "comb": combs2[c], "cnt1": cnt1,
            "vecs": vecs2, "w2": W2} for c in range(NCORES)]
    _t = _time.time()
    r3 = _run_retry(nc3, in3, list(range(NCORES)))
    LAST_TIMINGS["launch3"] = _time.time() - _t
    print("launch3 done", flush=True)

    out = np.zeros((N, C_OUT), np.float32)
    for c in range(NCORES):
        grid = cores_data[c]["grid"]
        valid = grid >= 0
        out[c * SH + grid[valid]] = r3.results[c]["y"][valid]
    LAST_INPUTS.update({"in1": in1, "in2": in2, "in3": in3})
    LAST_SCHED[0] = s
    return out
